# revision 16
# baseline (speedup 1.0000x reference)
"""PinSAGE-style sampled-neighbor mean + linear on 8 Trainium2 NeuronCores.

Strategy: shard the 100k nodes across 8 cores (12.5k each, 98 groups of
128); x stays replicated in HBM. The sampled-neighbor gather is the whole
problem: ~121k random 512B rows per core. Per-row indirect DMA is capped
at 128 descriptors / ~1us instruction (SWDGE fixed cost), so we use
dma_gather (InstDMAGatherAnt): ONE instruction per 4096 indices, each
index a descriptor (994ns + 0.34ns/desc). Its int16 index limit is beaten
by banking x into 4 slices of 25k rows and sorting each core's edge
references by (bank, node).

Gathered positions land slot-major (position i -> partition i%128, block
i//128). Reduction to per-node sums uses the PE: for each 128-slot tile,
build a one-hot selection matrix S[slot, node] = (nid[slot] == iota) on
DVE/GpSimd, then matmul(lhsT=X_tile, rhs=S) accumulated in PSUM over the
(4 banks x T tiles) of each node group -> aggT [feat, node]. Then one
matmul with W.T, scale by 1/c', add bias, stream out.

Per-(bank, group) cells are padded to a data-derived uniform tile count so
one compiled program serves all 8 SPMD cores; pad positions fetch bank row
0 and carry a sentinel node-id that matches nothing.
"""

import numpy as np

N_NODES = 100000
N_EDGES = 1600000
D = 128
TAPS = 10
N_CORES = 8
NODES_PC = 12500
G128 = 98                      # ceil(12500/128) groups of 128 nodes
NODES_PAD = G128 * 128         # 12544
BANKS = 4
BANK_ROWS = N_NODES // BANKS   # 25000 (< 32768 -> int16 indices)
BLK = 1024                     # gather positions per dma_gather (single-packet cap: 64 desc x 16 engines)
TPB = BLK // 128               # 32 tiles per gather block
OCH = 14                       # groups per output DMA chunk (98 = 7*14)
SENTINEL = 999.0

_cache = {}


def _build_refs(edge_index):
    """Kept-edge references with torch first-min(c,10) semantics.

    Returns cols [R] int64 (neighbor row per reference, node-major order),
    ref_node [R] int64 (global node of each reference), inv [N] f32 = 1/c'.
    Nodes with no out-edges get a single self reference.
    """
    row = np.asarray(edge_index[0], dtype=np.int64)
    col = np.asarray(edge_index[1], dtype=np.int64)
    E = row.shape[0]
    order = np.argsort(row, kind="stable")
    row_s = row[order]
    col_s = col[order]
    starts = np.searchsorted(row_s, np.arange(N_NODES, dtype=np.int64))
    counts = np.diff(np.append(starts, E))
    rank = np.arange(E, dtype=np.int64) - starts[row_s]
    keep = rank < TAPS
    kr = row_s[keep]
    kc = col_s[keep]
    self_nodes = np.nonzero(counts == 0)[0]
    ref_node = np.concatenate([kr, self_nodes])
    cols = np.concatenate([kc, self_nodes])
    o2 = np.argsort(ref_node, kind="stable")
    ref_node = ref_node[o2]
    cols = cols[o2]
    cnt_eff = np.maximum(np.minimum(counts, TAPS), 1)
    inv = (1.0 / cnt_eff).astype(np.float32)
    return cols, ref_node, inv


def _prep(x, edge_index, W, b):
    """Host prep: per-core gather/index tables. Returns (in_maps, cap_full,
    cap_last)."""
    x = np.ascontiguousarray(np.asarray(x, dtype=np.float32))
    W = np.asarray(W, dtype=np.float32)
    b = np.asarray(b, dtype=np.float32)

    cols, ref_node, inv = _build_refs(edge_index)
    core = ref_node // NODES_PC
    node_l = ref_node % NODES_PC
    bank = cols // BANK_ROWS
    col16 = (cols % BANK_ROWS).astype(np.int16)
    grp = node_l // 128

    # data-derived uniform cell capacities (same for every core -> SPMD)
    cell = ((core * BANKS + bank) * G128 + grp).astype(np.int64)
    cnts = np.bincount(cell, minlength=N_CORES * BANKS * G128)
    cnts = cnts.reshape(N_CORES, BANKS, G128)
    caps = np.maximum(cnts.max(axis=(0, 1)), 128).astype(np.int64)  # [G128]
    gstart = np.concatenate([[0], np.cumsum(caps)])
    p_bank = int(-(-gstart[-1] // 128) * 128)  # positions per bank stream
    caps_t = tuple(int(v) for v in caps)
    nb_b = -(-p_bank // BLK)                  # gather blocks per bank

    wt_host = np.ascontiguousarray(W.T)
    bias_host = np.ascontiguousarray(np.broadcast_to(b[None, :], (128, D)))
    iota_host = np.ascontiguousarray(
        np.broadcast_to(np.arange(256, dtype=np.float32)[None, :], (128, 256)))

    in_maps = []
    for c in range(N_CORES):
        m = core == c
        bk = bank[m]
        nl = node_l[m]
        c16 = col16[m]
        g = grp[m]
        # sort refs by (bank, node); compute position = cell_start + rank
        o = np.lexsort((nl, bk))
        bk, nl, c16, g = bk[o], nl[o], c16[o], g[o]
        cell_id = bk * G128 + g
        cell_start = bk * p_bank + gstart[g]
        ccnt = np.bincount(cell_id, minlength=BANKS * G128)
        first = np.concatenate([[0], np.cumsum(ccnt)[:-1]])
        rank = np.arange(bk.shape[0]) - first[cell_id]
        pos = cell_start + rank

        total = BANKS * p_bank
        idx16 = np.zeros(total, np.int16)
        nidf = np.full(total, SENTINEL, np.float32)
        idx16[pos] = c16
        pos_in_bank = pos - bk * p_bank
        tile_start = (pos_in_bank // 128) * 128
        primary = np.searchsorted(gstart, tile_start, side="right") - 1
        nidf[pos] = ((g - primary) * 128 + (nl % 128)).astype(np.float32)

        # wrap idx per gather block: elem i -> (partition i%16, col i//16),
        # replicated x8 to 128 partitions
        padded = BANKS * nb_b * BLK
        i16p = np.zeros(padded, np.int16)
        nfp = np.full(padded, SENTINEL, np.float32)
        for bb in range(BANKS):
            i16p[bb * nb_b * BLK: bb * nb_b * BLK + p_bank] = \
                idx16[bb * p_bank:(bb + 1) * p_bank]
            nfp[bb * nb_b * BLK: bb * nb_b * BLK + p_bank] = \
                nidf[bb * p_bank:(bb + 1) * p_bank]
        idxw = i16p.reshape(BANKS * nb_b, TPB * 8, 16).transpose(0, 2, 1)
        idxw = np.ascontiguousarray(
            np.tile(idxw, (1, 8, 1)))                      # [NB,128,256]
        nidw = np.ascontiguousarray(
            nfp.reshape(BANKS * nb_b, TPB, 128).transpose(0, 2, 1))

        inv_c = np.ones(NODES_PAD, np.float32)
        inv_c[:NODES_PC] = inv[c * NODES_PC:(c + 1) * NODES_PC]
        inv_sb = np.ascontiguousarray(
            inv_c.reshape(G128, 128).T)                    # [128, G128]

        in_maps.append({
            "x": x,
            "idxw": idxw,
            "nidt": nidw,
            "iota": iota_host,
            "inv": inv_sb,
            "wt": wt_host,
            "bias_rep": bias_host,
        })
    return in_maps, caps_t, p_bank


def _build_program(caps_t, p_bank):
    import concourse.bass as bass  # noqa: F401
    import concourse.mybir as mybir
    import concourse.tile as tile
    from concourse import bacc

    gstart = [0]
    for v in caps_t:
        gstart.append(gstart[-1] + v)

    def primary_of(tau):
        lo = tau * 128
        p = 0
        while gstart[p + 1] <= lo:
            p += 1
        return p

    nb_b = -(-p_bank // BLK)
    last_nidx = p_bank - (nb_b - 1) * BLK

    nc = bacc.Bacc("TRN2", target_bir_lowering=False, debug=False,
                   enable_asserts=True, num_devices=N_CORES)
    x = nc.dram_tensor("x", [N_NODES, D], mybir.dt.float32,
                       kind="ExternalInput").ap()
    idxw = nc.dram_tensor("idxw", [BANKS * nb_b, 128, TPB * 8],
                          mybir.dt.int16, kind="ExternalInput").ap()
    nidt = nc.dram_tensor("nidt", [BANKS * nb_b, 128, TPB],
                          mybir.dt.float32, kind="ExternalInput").ap()
    iota = nc.dram_tensor("iota", [128, 256], mybir.dt.float32,
                          kind="ExternalInput").ap()
    inv = nc.dram_tensor("inv", [128, G128], mybir.dt.float32,
                         kind="ExternalInput").ap()
    wt = nc.dram_tensor("wt", [D, D], mybir.dt.float32,
                        kind="ExternalInput").ap()
    bias_rep = nc.dram_tensor("bias_rep", [128, D], mybir.dt.float32,
                              kind="ExternalInput").ap()
    out = nc.dram_tensor("out", [NODES_PAD, D], mybir.dt.float32,
                         kind="ExternalOutput").ap()

    with tile.TileContext(nc) as tc:
        with tc.tile_pool(name="const", bufs=1) as const_p, \
             tc.tile_pool(name="idxp", bufs=8) as idx_p, \
             tc.tile_pool(name="nidp", bufs=8) as nid_p, \
             tc.tile_pool(name="gb0", bufs=2) as gp0, \
             tc.tile_pool(name="gb1", bufs=2) as gp1, \
             tc.tile_pool(name="gb2", bufs=2) as gp2, \
             tc.tile_pool(name="gb3", bufs=2) as gp3, \
             tc.tile_pool(name="sp", bufs=6) as s_p, \
             tc.tile_pool(name="stp", bufs=3) as st_p, \
             tc.tile_pool(name="outp", bufs=2) as out_p, \
             tc.tile_pool(name="ps1", bufs=2, space="PSUM") as ps1_p, \
             tc.tile_pool(name="ps2", bufs=2, space="PSUM") as ps2_p:
            gpools = [gp0, gp1, gp2, gp3]

            wt_sb = const_p.tile([D, D], mybir.dt.float32)
            nc.sync.dma_start(wt_sb[:], wt[:])
            bias_sb = const_p.tile([128, D], mybir.dt.float32)
            nc.sync.dma_start(bias_sb[:], bias_rep[:])
            iota_sb = const_p.tile([128, 256], mybir.dt.float32)
            nc.sync.dma_start(iota_sb[:], iota[:])
            inv_sb = const_p.tile([128, G128], mybir.dt.float32)
            nc.sync.dma_start(inv_sb[:], inv[:])

            issued = [0] * BANKS
            gtiles = {}
            ntiles = {}

            def ensure(b, blk):
                while issued[b] <= blk:
                    k = issued[b]
                    it = idx_p.tile([128, TPB * 8], mybir.dt.int16,
                                    name="idx_t")
                    nc.sync.dma_start(it[:], idxw[b * nb_b + k])
                    nt = nid_p.tile([128, TPB], mybir.dt.float32,
                                    name="nid_t")
                    nc.sync.dma_start(nt[:], nidt[b * nb_b + k])
                    G = gpools[b].tile([128, BLK], mybir.dt.float32,
                                       name=f"G{b}")
                    nidx = BLK if k < nb_b - 1 else last_nidx
                    nc.gpsimd.dma_gather(
                        out_ap=G[:, :nidx].rearrange("p (t d) -> p t d", d=D),
                        in_ap=x[b * BANK_ROWS:(b + 1) * BANK_ROWS, :],
                        idxs_ap=it[:],
                        num_idxs=nidx,
                        num_idxs_reg=nidx,
                        elem_size=D,
                    )
                    gtiles[(b, k)] = G
                    ntiles[(b, k)] = nt
                    issued[b] += 1

            sctr = 0
            o_sb = None
            import os
            glim = int(os.environ.get("KERN_GLIM", G128))
            for g in range(glim):
                fg = gstart[g] // 128
                lg = (gstart[g + 1] - 1) // 128
                T = lg - fg + 1
                ps = ps1_p.tile([128, 128], mybir.dt.float32, space="PSUM",
                                name="ps1")
                nmm = BANKS * T
                mi = 0
                for b in range(BANKS):
                    for tau in range(fg, lg + 1):
                        blk, tcol = tau // TPB, tau % TPB
                        ensure(b, blk)
                        if blk + 1 < nb_b:
                            ensure(b, blk + 1)   # prefetch
                        prim = primary_of(tau)
                        off = (g - prim) * 128
                        w = off + 128
                        S = s_p.tile([128, 256], mybir.dt.float32, name="S")
                        eng = nc.vector
                        sctr += 1
                        eng.tensor_scalar(
                            out=S[:, :w],
                            in0=iota_sb[:, :w],
                            scalar1=ntiles[(b, blk)][:, tcol:tcol + 1],
                            scalar2=None,
                            op0=mybir.AluOpType.is_equal,
                        )
                        nc.tensor.matmul(
                            ps[:],
                            lhsT=gtiles[(b, blk)][:, tcol * 128:(tcol + 1) * 128],
                            rhs=S[:, off:off + 128],
                            start=(mi == 0),
                            stop=(mi == nmm - 1),
                        )
                        mi += 1
                sT = st_p.tile([128, 128], mybir.dt.float32, name="sT")
                nc.scalar.copy(sT[:], ps[:])
                ps2 = ps2_p.tile([128, 128], mybir.dt.float32, space="PSUM",
                                 name="ps2")
                nc.tensor.matmul(ps2[:], lhsT=sT[:], rhs=wt_sb[:],
                                 start=True, stop=True)
                if g % OCH == 0:
                    o_sb = out_p.tile([128, OCH * D], mybir.dt.float32,
                                      name="o_sb")
                gl = g % OCH
                nc.vector.scalar_tensor_tensor(
                    out=o_sb[:, gl * D:(gl + 1) * D],
                    in0=ps2[:],
                    scalar=inv_sb[:, g:g + 1],
                    in1=bias_sb[:],
                    op0=mybir.AluOpType.mult,
                    op1=mybir.AluOpType.add,
                )
                if gl == OCH - 1:
                    g0 = g - (OCH - 1)
                    nc.sync.dma_start(
                        out[g0 * 128:(g0 + OCH) * 128, :]
                        .rearrange("(c p) d -> p c d", p=128),
                        o_sb[:].rearrange("p (c d) -> p c d", c=OCH),
                    )
    nc.compile()
    return nc


def kernel(x, edge_index, W, b):
    from concourse.bass_utils import run_bass_kernel_spmd

    in_maps, caps_t, p_bank = _prep(x, edge_index, W, b)

    import os
    key = ("nc", caps_t, p_bank, os.environ.get("KERN_GLIM", ""))
    if key not in _cache:
        _cache[key] = _build_program(caps_t, p_bank)
    nc = _cache[key]
    _cache["nc"] = nc  # for test harness reuse

    res = run_bass_kernel_spmd(nc, in_maps, core_ids=list(range(N_CORES)))
    outs = [res.results[c]["out"][:NODES_PC] for c in range(N_CORES)]
    return np.concatenate(outs, axis=0)


# revision 18
# speedup vs baseline: 1.0205x; 1.0205x over previous
"""PinSAGE-style sampled-neighbor mean + linear on 8 Trainium2 NeuronCores.

Strategy: shard the 100k nodes across 8 cores (12.5k each, 98 groups of
128); x stays replicated in HBM. The sampled-neighbor gather is the whole
problem: ~121k random 512B rows per core. Per-row indirect DMA is capped
at 128 descriptors / ~1us instruction (SWDGE fixed cost), so we use
dma_gather (InstDMAGatherAnt): ONE instruction per 4096 indices, each
index a descriptor (994ns + 0.34ns/desc). Its int16 index limit is beaten
by banking x into 4 slices of 25k rows and sorting each core's edge
references by (bank, node).

Gathered positions land slot-major (position i -> partition i%128, block
i//128). Reduction to per-node sums uses the PE: for each 128-slot tile,
build a one-hot selection matrix S[slot, node] = (nid[slot] == iota) on
DVE/GpSimd, then matmul(lhsT=X_tile, rhs=S) accumulated in PSUM over the
(4 banks x T tiles) of each node group -> aggT [feat, node]. Then one
matmul with W.T, scale by 1/c', add bias, stream out.

Per-(bank, group) cells are padded to a data-derived uniform tile count so
one compiled program serves all 8 SPMD cores; pad positions fetch bank row
0 and carry a sentinel node-id that matches nothing.
"""

import numpy as np

N_NODES = 100000
N_EDGES = 1600000
D = 128
TAPS = 10
N_CORES = 8
NODES_PC = 12500
G128 = 98                      # ceil(12500/128) groups of 128 nodes
NODES_PAD = G128 * 128         # 12544
BANKS = 4
BANK_ROWS = N_NODES // BANKS   # 25000 (< 32768 -> int16 indices)
BLK = 1024                     # gather positions per dma_gather (single-packet cap: 64 desc x 16 engines)
TPB = BLK // 128               # 32 tiles per gather block
OCH = 14                       # groups per output DMA chunk (98 = 7*14)
SENTINEL = 999.0

_cache = {}


def _build_refs(edge_index):
    """Kept-edge references with torch first-min(c,10) semantics.

    Returns cols [R] int64 (neighbor row per reference, node-major order),
    ref_node [R] int64 (global node of each reference), inv [N] f32 = 1/c'.
    Nodes with no out-edges get a single self reference.
    """
    row = np.asarray(edge_index[0], dtype=np.int64)
    col = np.asarray(edge_index[1], dtype=np.int64)
    E = row.shape[0]
    order = np.argsort(row, kind="stable")
    row_s = row[order]
    col_s = col[order]
    starts = np.searchsorted(row_s, np.arange(N_NODES, dtype=np.int64))
    counts = np.diff(np.append(starts, E))
    rank = np.arange(E, dtype=np.int64) - starts[row_s]
    keep = rank < TAPS
    kr = row_s[keep]
    kc = col_s[keep]
    self_nodes = np.nonzero(counts == 0)[0]
    ref_node = np.concatenate([kr, self_nodes])
    cols = np.concatenate([kc, self_nodes])
    o2 = np.argsort(ref_node, kind="stable")
    ref_node = ref_node[o2]
    cols = cols[o2]
    cnt_eff = np.maximum(np.minimum(counts, TAPS), 1)
    inv = (1.0 / cnt_eff).astype(np.float32)
    return cols, ref_node, inv


def _prep(x, edge_index, W, b):
    """Host prep: per-core gather/index tables. Returns (in_maps, cap_full,
    cap_last)."""
    x = np.ascontiguousarray(np.asarray(x, dtype=np.float32))
    W = np.asarray(W, dtype=np.float32)
    b = np.asarray(b, dtype=np.float32)

    cols, ref_node, inv = _build_refs(edge_index)
    core = ref_node // NODES_PC
    node_l = ref_node % NODES_PC
    bank = cols // BANK_ROWS
    col16 = (cols % BANK_ROWS).astype(np.int16)
    grp = node_l // 128

    # data-derived uniform cell capacities (same for every core -> SPMD)
    cell = ((core * BANKS + bank) * G128 + grp).astype(np.int64)
    cnts = np.bincount(cell, minlength=N_CORES * BANKS * G128)
    cnts = cnts.reshape(N_CORES, BANKS, G128)
    cap_full = int(np.ceil(cnts[:, :, :G128 - 1].max() / 128) * 128)
    cap_last = int(np.ceil(max(cnts[:, :, G128 - 1].max(), 1) / 128) * 128)
    t_full = cap_full // 128
    t_last = cap_last // 128
    tiles_ps = (G128 - 1) * t_full + t_last   # tiles per bank stream
    p_bank = tiles_ps * 128                   # positions per bank stream
    nb_b = -(-p_bank // BLK)                  # gather blocks per bank

    wt_host = np.ascontiguousarray(W.T)
    bias_host = np.ascontiguousarray(np.broadcast_to(b[None, :], (128, D)))
    iota_host = np.ascontiguousarray(
        np.broadcast_to(np.arange(128, dtype=np.float32)[None, :], (128, 128)))

    in_maps = []
    for c in range(N_CORES):
        m = core == c
        bk = bank[m]
        nl = node_l[m]
        c16 = col16[m]
        g = grp[m]
        # sort refs by (bank, node); compute position = cell_start + rank
        o = np.lexsort((nl, bk))
        bk, nl, c16, g = bk[o], nl[o], c16[o], g[o]
        cell_id = bk * G128 + g
        cell_start = (bk * p_bank + np.minimum(g, G128 - 1) * cap_full)
        ccnt = np.bincount(cell_id, minlength=BANKS * G128)
        first = np.concatenate([[0], np.cumsum(ccnt)[:-1]])
        rank = np.arange(bk.shape[0]) - first[cell_id]
        pos = cell_start + rank

        total = BANKS * p_bank
        idx16 = np.zeros(total, np.int16)
        nidf = np.full(total, SENTINEL, np.float32)
        idx16[pos] = c16
        nidf[pos] = (nl % 128).astype(np.float32)

        # wrap idx per gather block: elem i -> (partition i%16, col i//16),
        # replicated x8 to 128 partitions
        padded = BANKS * nb_b * BLK
        i16p = np.zeros(padded, np.int16)
        nfp = np.full(padded, SENTINEL, np.float32)
        for bb in range(BANKS):
            i16p[bb * nb_b * BLK: bb * nb_b * BLK + p_bank] = \
                idx16[bb * p_bank:(bb + 1) * p_bank]
            nfp[bb * nb_b * BLK: bb * nb_b * BLK + p_bank] = \
                nidf[bb * p_bank:(bb + 1) * p_bank]
        idxw = i16p.reshape(BANKS * nb_b, TPB * 8, 16).transpose(0, 2, 1)
        idxw = np.ascontiguousarray(
            np.tile(idxw, (1, 8, 1)))                      # [NB,128,256]
        nidw = np.ascontiguousarray(
            nfp.reshape(BANKS * nb_b, TPB, 128).transpose(0, 2, 1))

        inv_c = np.ones(NODES_PAD, np.float32)
        inv_c[:NODES_PC] = inv[c * NODES_PC:(c + 1) * NODES_PC]
        inv_sb = np.ascontiguousarray(
            inv_c.reshape(G128, 128).T)                    # [128, G128]

        in_maps.append({
            "x": x,
            "idxw": idxw,
            "nidt": nidw,
            "iota": iota_host,
            "inv": inv_sb,
            "wt": wt_host,
            "bias_rep": bias_host,
        })
    return in_maps, cap_full, cap_last


def _build_program(cap_full, cap_last):
    import concourse.bass as bass  # noqa: F401
    import concourse.mybir as mybir
    import concourse.tile as tile
    from concourse import bacc

    t_full = cap_full // 128
    t_last = cap_last // 128
    tiles_ps = (G128 - 1) * t_full + t_last
    p_bank = tiles_ps * 128
    nb_b = -(-p_bank // BLK)
    last_nidx = p_bank - (nb_b - 1) * BLK

    nc = bacc.Bacc("TRN2", target_bir_lowering=False, debug=False,
                   enable_asserts=True, num_devices=N_CORES,
                   dynamic_dma_scratch_size=65536)
    x = nc.dram_tensor("x", [N_NODES, D], mybir.dt.float32,
                       kind="ExternalInput").ap()
    idxw = nc.dram_tensor("idxw", [BANKS * nb_b, 128, TPB * 8],
                          mybir.dt.int16, kind="ExternalInput").ap()
    nidt = nc.dram_tensor("nidt", [BANKS * nb_b, 128, TPB],
                          mybir.dt.float32, kind="ExternalInput").ap()
    iota = nc.dram_tensor("iota", [128, 128], mybir.dt.float32,
                          kind="ExternalInput").ap()
    inv = nc.dram_tensor("inv", [128, G128], mybir.dt.float32,
                         kind="ExternalInput").ap()
    wt = nc.dram_tensor("wt", [D, D], mybir.dt.float32,
                        kind="ExternalInput").ap()
    bias_rep = nc.dram_tensor("bias_rep", [128, D], mybir.dt.float32,
                              kind="ExternalInput").ap()
    out = nc.dram_tensor("out", [NODES_PAD, D], mybir.dt.float32,
                         kind="ExternalOutput").ap()

    with tile.TileContext(nc) as tc:
        with tc.tile_pool(name="const", bufs=1) as const_p, \
             tc.tile_pool(name="idxp", bufs=8) as idx_p, \
             tc.tile_pool(name="nidp", bufs=8) as nid_p, \
             tc.tile_pool(name="gb0", bufs=2) as gp0, \
             tc.tile_pool(name="gb1", bufs=2) as gp1, \
             tc.tile_pool(name="gb2", bufs=2) as gp2, \
             tc.tile_pool(name="gb3", bufs=2) as gp3, \
             tc.tile_pool(name="sp", bufs=6) as s_p, \
             tc.tile_pool(name="stp", bufs=3) as st_p, \
             tc.tile_pool(name="outp", bufs=2) as out_p, \
             tc.tile_pool(name="ps1", bufs=2, space="PSUM") as ps1_p, \
             tc.tile_pool(name="ps2", bufs=2, space="PSUM") as ps2_p:
            gpools = [gp0, gp1, gp2, gp3]

            wt_sb = const_p.tile([D, D], mybir.dt.float32)
            nc.sync.dma_start(wt_sb[:], wt[:])
            bias_sb = const_p.tile([128, D], mybir.dt.float32)
            nc.sync.dma_start(bias_sb[:], bias_rep[:])
            iota_sb = const_p.tile([128, 128], mybir.dt.float32)
            nc.sync.dma_start(iota_sb[:], iota[:])
            inv_sb = const_p.tile([128, G128], mybir.dt.float32)
            nc.sync.dma_start(inv_sb[:], inv[:])

            issued = [0] * BANKS
            gtiles = {}
            ntiles = {}

            def ensure(b, blk):
                while issued[b] <= blk:
                    k = issued[b]
                    it = idx_p.tile([128, TPB * 8], mybir.dt.int16,
                                    name="idx_t")
                    nc.sync.dma_start(it[:], idxw[b * nb_b + k])
                    nt = nid_p.tile([128, TPB], mybir.dt.float32,
                                    name="nid_t")
                    nc.sync.dma_start(nt[:], nidt[b * nb_b + k])
                    G = gpools[b].tile([128, BLK], mybir.dt.float32,
                                       name=f"G{b}")
                    nidx = BLK if k < nb_b - 1 else last_nidx
                    nc.gpsimd.dma_gather(
                        out_ap=G[:, :nidx].rearrange("p (t d) -> p t d", d=D),
                        in_ap=x[b * BANK_ROWS:(b + 1) * BANK_ROWS, :],
                        idxs_ap=it[:],
                        num_idxs=nidx,
                        num_idxs_reg=nidx,
                        elem_size=D,
                    )
                    gtiles[(b, k)] = G
                    ntiles[(b, k)] = nt
                    issued[b] += 1

            sctr = 0
            o_sb = None
            import os
            glim = int(os.environ.get("KERN_GLIM", G128))
            for g in range(glim):
                T = t_full if g < G128 - 1 else t_last
                ps = ps1_p.tile([128, 128], mybir.dt.float32, space="PSUM",
                                name="ps1")
                nmm = BANKS * T
                mi = 0
                for b in range(BANKS):
                    for t in range(T):
                        tau = g * t_full + t
                        blk, tcol = tau // TPB, tau % TPB
                        ensure(b, blk)
                        if blk + 1 < nb_b:
                            ensure(b, blk + 1)   # prefetch
                        S = s_p.tile([128, 128], mybir.dt.float32, name="S")
                        eng = nc.vector
                        sctr += 1
                        eng.tensor_scalar(
                            out=S[:],
                            in0=iota_sb[:],
                            scalar1=ntiles[(b, blk)][:, tcol:tcol + 1],
                            scalar2=None,
                            op0=mybir.AluOpType.is_equal,
                        )
                        nc.tensor.matmul(
                            ps[:],
                            lhsT=gtiles[(b, blk)][:, tcol * 128:(tcol + 1) * 128],
                            rhs=S[:],
                            start=(mi == 0),
                            stop=(mi == nmm - 1),
                        )
                        mi += 1
                sT = st_p.tile([128, 128], mybir.dt.float32, name="sT")
                nc.scalar.copy(sT[:], ps[:])
                ps2 = ps2_p.tile([128, 128], mybir.dt.float32, space="PSUM",
                                 name="ps2")
                nc.tensor.matmul(ps2[:], lhsT=sT[:], rhs=wt_sb[:],
                                 start=True, stop=True)
                if g % OCH == 0:
                    o_sb = out_p.tile([128, OCH * D], mybir.dt.float32,
                                      name="o_sb")
                gl = g % OCH
                nc.vector.scalar_tensor_tensor(
                    out=o_sb[:, gl * D:(gl + 1) * D],
                    in0=ps2[:],
                    scalar=inv_sb[:, g:g + 1],
                    in1=bias_sb[:],
                    op0=mybir.AluOpType.mult,
                    op1=mybir.AluOpType.add,
                )
                if gl == OCH - 1:
                    g0 = g - (OCH - 1)
                    nc.sync.dma_start(
                        out[g0 * 128:(g0 + OCH) * 128, :]
                        .rearrange("(c p) d -> p c d", p=128),
                        o_sb[:].rearrange("p (c d) -> p c d", c=OCH),
                    )
    nc.compile()
    return nc


def kernel(x, edge_index, W, b):
    from concourse.bass_utils import run_bass_kernel_spmd

    in_maps, cap_full, cap_last = _prep(x, edge_index, W, b)

    import os
    key = ("nc", cap_full, cap_last, os.environ.get("KERN_GLIM", ""))
    if key not in _cache:
        _cache[key] = _build_program(cap_full, cap_last)
    nc = _cache[key]
    _cache["nc"] = nc  # for test harness reuse

    res = run_bass_kernel_spmd(nc, in_maps, core_ids=list(range(N_CORES)))
    outs = [res.results[c]["out"][:NODES_PC] for c in range(N_CORES)]
    return np.concatenate(outs, axis=0)


# revision 19
# speedup vs baseline: 1.0222x; 1.0017x over previous
"""PinSAGE-style sampled-neighbor mean + linear on 8 Trainium2 NeuronCores.

Strategy: shard the 100k nodes across 8 cores (12.5k each, 98 groups of
128); x stays replicated in HBM. The sampled-neighbor gather is the whole
problem: ~121k random 512B rows per core. Per-row indirect DMA is capped
at 128 descriptors / ~1us instruction (SWDGE fixed cost), so we use
dma_gather (InstDMAGatherAnt): ONE instruction per 4096 indices, each
index a descriptor (994ns + 0.34ns/desc). Its int16 index limit is beaten
by banking x into 4 slices of 25k rows and sorting each core's edge
references by (bank, node).

Gathered positions land slot-major (position i -> partition i%128, block
i//128). Reduction to per-node sums uses the PE: for each 128-slot tile,
build a one-hot selection matrix S[slot, node] = (nid[slot] == iota) on
DVE/GpSimd, then matmul(lhsT=X_tile, rhs=S) accumulated in PSUM over the
(4 banks x T tiles) of each node group -> aggT [feat, node]. Then one
matmul with W.T, scale by 1/c', add bias, stream out.

Per-(bank, group) cells are padded to a data-derived uniform tile count so
one compiled program serves all 8 SPMD cores; pad positions fetch bank row
0 and carry a sentinel node-id that matches nothing.
"""

import numpy as np

N_NODES = 100000
N_EDGES = 1600000
D = 128
TAPS = 10
N_CORES = 8
NODES_PC = 12500
G128 = 98                      # ceil(12500/128) groups of 128 nodes
NODES_PAD = G128 * 128         # 12544
BANKS = 4
BANK_ROWS = N_NODES // BANKS   # 25000 (< 32768 -> int16 indices)
BLK = 1024                     # gather positions per dma_gather (single-packet cap: 64 desc x 16 engines)
TPB = BLK // 128               # 32 tiles per gather block
OCH = 14                       # groups per output DMA chunk (98 = 7*14)
SENTINEL = 999.0

_cache = {}


def _build_refs(edge_index):
    """Kept-edge references with torch first-min(c,10) semantics.

    Returns cols [R] int64 (neighbor row per reference, node-major order),
    ref_node [R] int64 (global node of each reference), inv [N] f32 = 1/c'.
    Nodes with no out-edges get a single self reference.
    """
    row = np.asarray(edge_index[0], dtype=np.int64)
    col = np.asarray(edge_index[1], dtype=np.int64)
    E = row.shape[0]
    order = np.argsort(row, kind="stable")
    row_s = row[order]
    col_s = col[order]
    starts = np.searchsorted(row_s, np.arange(N_NODES, dtype=np.int64))
    counts = np.diff(np.append(starts, E))
    rank = np.arange(E, dtype=np.int64) - starts[row_s]
    keep = rank < TAPS
    kr = row_s[keep]
    kc = col_s[keep]
    self_nodes = np.nonzero(counts == 0)[0]
    ref_node = np.concatenate([kr, self_nodes])
    cols = np.concatenate([kc, self_nodes])
    o2 = np.argsort(ref_node, kind="stable")
    ref_node = ref_node[o2]
    cols = cols[o2]
    cnt_eff = np.maximum(np.minimum(counts, TAPS), 1)
    inv = (1.0 / cnt_eff).astype(np.float32)
    return cols, ref_node, inv


def _prep(x, edge_index, W, b):
    """Host prep: per-core gather/index tables. Returns (in_maps, cap_full,
    cap_last)."""
    x = np.ascontiguousarray(np.asarray(x, dtype=np.float32))
    W = np.asarray(W, dtype=np.float32)
    b = np.asarray(b, dtype=np.float32)

    cols, ref_node, inv = _build_refs(edge_index)
    core = ref_node // NODES_PC
    node_l = ref_node % NODES_PC
    bank = cols // BANK_ROWS
    col16 = (cols % BANK_ROWS).astype(np.int16)
    grp = node_l // 128

    # data-derived uniform cell capacities (same for every core -> SPMD)
    cell = ((core * BANKS + bank) * G128 + grp).astype(np.int64)
    cnts = np.bincount(cell, minlength=N_CORES * BANKS * G128)
    cnts = cnts.reshape(N_CORES, BANKS, G128)
    cap_full = int(np.ceil(cnts[:, :, :G128 - 1].max() / 128) * 128)
    cap_last = int(np.ceil(max(cnts[:, :, G128 - 1].max(), 1) / 128) * 128)
    t_full = cap_full // 128
    t_last = cap_last // 128
    tiles_ps = (G128 - 1) * t_full + t_last   # tiles per bank stream
    p_bank = tiles_ps * 128                   # positions per bank stream
    nb_b = -(-p_bank // BLK)                  # gather blocks per bank

    wt_host = np.ascontiguousarray(W.T)
    bias_host = np.ascontiguousarray(np.broadcast_to(b[None, :], (128, D)))
    iota_host = np.ascontiguousarray(
        np.broadcast_to(np.arange(128, dtype=np.float32)[None, :], (128, 128)))

    in_maps = []
    for c in range(N_CORES):
        m = core == c
        bk = bank[m]
        nl = node_l[m]
        c16 = col16[m]
        g = grp[m]
        # sort refs by (bank, group, col): cell grouping for the schedule,
        # ascending columns within each cell for HBM row-buffer locality
        o = np.lexsort((c16, g, bk))
        bk, nl, c16, g = bk[o], nl[o], c16[o], g[o]
        cell_id = bk * G128 + g
        cell_start = (bk * p_bank + np.minimum(g, G128 - 1) * cap_full)
        ccnt = np.bincount(cell_id, minlength=BANKS * G128)
        first = np.concatenate([[0], np.cumsum(ccnt)[:-1]])
        rank = np.arange(bk.shape[0]) - first[cell_id]
        pos = cell_start + rank

        total = BANKS * p_bank
        idx16 = np.zeros(total, np.int16)
        nidf = np.full(total, SENTINEL, np.float32)
        idx16[pos] = c16
        nidf[pos] = (nl % 128).astype(np.float32)

        # wrap idx per gather block: elem i -> (partition i%16, col i//16),
        # replicated x8 to 128 partitions
        padded = BANKS * nb_b * BLK
        i16p = np.zeros(padded, np.int16)
        nfp = np.full(padded, SENTINEL, np.float32)
        for bb in range(BANKS):
            i16p[bb * nb_b * BLK: bb * nb_b * BLK + p_bank] = \
                idx16[bb * p_bank:(bb + 1) * p_bank]
            nfp[bb * nb_b * BLK: bb * nb_b * BLK + p_bank] = \
                nidf[bb * p_bank:(bb + 1) * p_bank]
        idxw = i16p.reshape(BANKS * nb_b, TPB * 8, 16).transpose(0, 2, 1)
        idxw = np.ascontiguousarray(
            np.tile(idxw, (1, 8, 1)))                      # [NB,128,256]
        nidw = np.ascontiguousarray(
            nfp.reshape(BANKS * nb_b, TPB, 128).transpose(0, 2, 1))

        inv_c = np.ones(NODES_PAD, np.float32)
        inv_c[:NODES_PC] = inv[c * NODES_PC:(c + 1) * NODES_PC]
        inv_sb = np.ascontiguousarray(
            inv_c.reshape(G128, 128).T)                    # [128, G128]

        in_maps.append({
            "x": x,
            "idxw": idxw,
            "nidt": nidw,
            "iota": iota_host,
            "inv": inv_sb,
            "wt": wt_host,
            "bias_rep": bias_host,
        })
    return in_maps, cap_full, cap_last


def _build_program(cap_full, cap_last):
    import concourse.bass as bass  # noqa: F401
    import concourse.mybir as mybir
    import concourse.tile as tile
    from concourse import bacc

    t_full = cap_full // 128
    t_last = cap_last // 128
    tiles_ps = (G128 - 1) * t_full + t_last
    p_bank = tiles_ps * 128
    nb_b = -(-p_bank // BLK)
    last_nidx = p_bank - (nb_b - 1) * BLK

    nc = bacc.Bacc("TRN2", target_bir_lowering=False, debug=False,
                   enable_asserts=True, num_devices=N_CORES,
                   dynamic_dma_scratch_size=65536)
    x = nc.dram_tensor("x", [N_NODES, D], mybir.dt.float32,
                       kind="ExternalInput").ap()
    idxw = nc.dram_tensor("idxw", [BANKS * nb_b, 128, TPB * 8],
                          mybir.dt.int16, kind="ExternalInput").ap()
    nidt = nc.dram_tensor("nidt", [BANKS * nb_b, 128, TPB],
                          mybir.dt.float32, kind="ExternalInput").ap()
    iota = nc.dram_tensor("iota", [128, 128], mybir.dt.float32,
                          kind="ExternalInput").ap()
    inv = nc.dram_tensor("inv", [128, G128], mybir.dt.float32,
                         kind="ExternalInput").ap()
    wt = nc.dram_tensor("wt", [D, D], mybir.dt.float32,
                        kind="ExternalInput").ap()
    bias_rep = nc.dram_tensor("bias_rep", [128, D], mybir.dt.float32,
                              kind="ExternalInput").ap()
    out = nc.dram_tensor("out", [NODES_PAD, D], mybir.dt.float32,
                         kind="ExternalOutput").ap()

    with tile.TileContext(nc) as tc:
        with tc.tile_pool(name="const", bufs=1) as const_p, \
             tc.tile_pool(name="idxp", bufs=8) as idx_p, \
             tc.tile_pool(name="nidp", bufs=8) as nid_p, \
             tc.tile_pool(name="gb0", bufs=2) as gp0, \
             tc.tile_pool(name="gb1", bufs=2) as gp1, \
             tc.tile_pool(name="gb2", bufs=2) as gp2, \
             tc.tile_pool(name="gb3", bufs=2) as gp3, \
             tc.tile_pool(name="sp", bufs=6) as s_p, \
             tc.tile_pool(name="stp", bufs=3) as st_p, \
             tc.tile_pool(name="outp", bufs=2) as out_p, \
             tc.tile_pool(name="ps1", bufs=2, space="PSUM") as ps1_p, \
             tc.tile_pool(name="ps2", bufs=2, space="PSUM") as ps2_p:
            gpools = [gp0, gp1, gp2, gp3]

            wt_sb = const_p.tile([D, D], mybir.dt.float32)
            nc.sync.dma_start(wt_sb[:], wt[:])
            bias_sb = const_p.tile([128, D], mybir.dt.float32)
            nc.sync.dma_start(bias_sb[:], bias_rep[:])
            iota_sb = const_p.tile([128, 128], mybir.dt.float32)
            nc.sync.dma_start(iota_sb[:], iota[:])
            inv_sb = const_p.tile([128, G128], mybir.dt.float32)
            nc.sync.dma_start(inv_sb[:], inv[:])

            issued = [0] * BANKS
            gtiles = {}
            ntiles = {}

            def ensure(b, blk):
                while issued[b] <= blk:
                    k = issued[b]
                    it = idx_p.tile([128, TPB * 8], mybir.dt.int16,
                                    name="idx_t")
                    nc.sync.dma_start(it[:], idxw[b * nb_b + k])
                    nt = nid_p.tile([128, TPB], mybir.dt.float32,
                                    name="nid_t")
                    nc.sync.dma_start(nt[:], nidt[b * nb_b + k])
                    G = gpools[b].tile([128, BLK], mybir.dt.float32,
                                       name=f"G{b}")
                    nidx = BLK if k < nb_b - 1 else last_nidx
                    nc.gpsimd.dma_gather(
                        out_ap=G[:, :nidx].rearrange("p (t d) -> p t d", d=D),
                        in_ap=x[b * BANK_ROWS:(b + 1) * BANK_ROWS, :],
                        idxs_ap=it[:],
                        num_idxs=nidx,
                        num_idxs_reg=nidx,
                        elem_size=D,
                    )
                    gtiles[(b, k)] = G
                    ntiles[(b, k)] = nt
                    issued[b] += 1

            sctr = 0
            o_sb = None
            import os
            glim = int(os.environ.get("KERN_GLIM", G128))
            for g in range(glim):
                T = t_full if g < G128 - 1 else t_last
                ps = ps1_p.tile([128, 128], mybir.dt.float32, space="PSUM",
                                name="ps1")
                nmm = BANKS * T
                mi = 0
                for b in range(BANKS):
                    for t in range(T):
                        tau = g * t_full + t
                        blk, tcol = tau // TPB, tau % TPB
                        ensure(b, blk)
                        if blk + 1 < nb_b:
                            ensure(b, blk + 1)   # prefetch
                        S = s_p.tile([128, 128], mybir.dt.float32, name="S")
                        eng = nc.vector
                        sctr += 1
                        eng.tensor_scalar(
                            out=S[:],
                            in0=iota_sb[:],
                            scalar1=ntiles[(b, blk)][:, tcol:tcol + 1],
                            scalar2=None,
                            op0=mybir.AluOpType.is_equal,
                        )
                        nc.tensor.matmul(
                            ps[:],
                            lhsT=gtiles[(b, blk)][:, tcol * 128:(tcol + 1) * 128],
                            rhs=S[:],
                            start=(mi == 0),
                            stop=(mi == nmm - 1),
                        )
                        mi += 1
                sT = st_p.tile([128, 128], mybir.dt.float32, name="sT")
                nc.scalar.copy(sT[:], ps[:])
                ps2 = ps2_p.tile([128, 128], mybir.dt.float32, space="PSUM",
                                 name="ps2")
                nc.tensor.matmul(ps2[:], lhsT=sT[:], rhs=wt_sb[:],
                                 start=True, stop=True)
                if g % OCH == 0:
                    o_sb = out_p.tile([128, OCH * D], mybir.dt.float32,
                                      name="o_sb")
                gl = g % OCH
                nc.vector.scalar_tensor_tensor(
                    out=o_sb[:, gl * D:(gl + 1) * D],
                    in0=ps2[:],
                    scalar=inv_sb[:, g:g + 1],
                    in1=bias_sb[:],
                    op0=mybir.AluOpType.mult,
                    op1=mybir.AluOpType.add,
                )
                if gl == OCH - 1:
                    g0 = g - (OCH - 1)
                    nc.sync.dma_start(
                        out[g0 * 128:(g0 + OCH) * 128, :]
                        .rearrange("(c p) d -> p c d", p=128),
                        o_sb[:].rearrange("p (c d) -> p c d", c=OCH),
                    )
    nc.compile()
    return nc


def kernel(x, edge_index, W, b):
    from concourse.bass_utils import run_bass_kernel_spmd

    in_maps, cap_full, cap_last = _prep(x, edge_index, W, b)

    import os
    key = ("nc", cap_full, cap_last, os.environ.get("KERN_GLIM", ""))
    if key not in _cache:
        _cache[key] = _build_program(cap_full, cap_last)
    nc = _cache[key]
    _cache["nc"] = nc  # for test harness reuse

    res = run_bass_kernel_spmd(nc, in_maps, core_ids=list(range(N_CORES)))
    outs = [res.results[c]["out"][:NODES_PC] for c in range(N_CORES)]
    return np.concatenate(outs, axis=0)
